# revision 30
# baseline (speedup 1.0000x reference)
"""Trainium2 Bass kernel for the AgentLoss problem (raw bacc, manual sems).

Math: for each (l, b) the reference computes the masked cosine-similarity sum
    S = sum_{i != j} <x_i, x_j> / (|x_i| |x_j| + EPS)
over n=1024 agents with c=64 channels, then loss = sum_l mean_b S / (n(n-1)).

Since EPS (1e-5) is tiny vs |x_i||x_j| ~ 64, expand
    1/(m_i m_j + EPS) = r_i r_j - EPS r_i^2 r_j^2 + O(EPS^2),  r_i = 1/m_i
which makes the double sum separable:
    S ~= (|sum_i x_i r_i|^2 - sum_i msq_i r_i^2)
         - EPS * (|sum_i x_i r_i^2|^2 - sum_i msq_i r_i^4)
with sum_i msq_i r_i^2 ~= n (fp32 recip, exact to ~1e-7) and
sum_i msq_i r_i^4 ~= sum_i r_i^2 (= t2, via a ones-stationary matmul).

The device side runs in bf16: the host pre-casts the input (cosine
similarity is scale-free and the loss averages ~16M sims, so the cast
costs ~3e-3 relative error - well under the 2e-2 gate), which halves HBM
traffic and lets the PE stream matmuls at full rate instead of fp32's
LOW/HIGH half-rate split.  Pipeline per (l, b) pair:

  in-DMA (4 chunks, sizes 1/2/2/3 pairs, one sem each - per-chunk sems are
  required because concurrent DMAs interleave their 16 per-engine sem incs)
  -> square: ACT pairs {0,1,3,5,7} / GpSimd {2,4,6} (xsq fp32)
  -> segmented reduce to per-agent msq: DVE tensor_reduce (the critical
     ~5.4us chain; nothing else on the chip can reduce a free axis)
  -> r^2 = 1/msq: DVE RECIPROCAL_APPROX_FAST custom op (~51 ULP, 5x faster
     than iterative divide), fp32 into rsq
  -> weights: ACT sqrt casts r to bf16, GpSimd copy casts r^2 to bf16,
     both into the [tt, (r,r,r2,r2)] stationary tile W
  -> thin bf16 matmuls contract the agent axis, 2 sub-rows x {r, r^2} per
     matmul (N=128 moving, half-garbage output the host discards); pairs
     6/7 write separate PSUM banks so the last staging copies wait only on
     their own pair (reading a bank that another accumulation group is
     mid-flight in is an NRT_EXEC_UNIT_UNRECOVERABLE on HW)
  -> staging copies split ACT/DVE, 2 out-DMAs.

Groups (2,2,2,1,1) drain the recip/sqrt ladder through single pairs at the
end.  A dummy sqrt up front pulls the ACT table load into the DMA phase.
No final receipt wait or semaphore clears: the framework postamble clears
all 253 sems (~7us) after the out-DMA receipt lands, giving the write a
multi-microsecond margin before stream end.  Host does the final ~2k-flop
combine in float64.

Sharding: data-parallel over batch b - core k takes b in {2k, 2k+1}, i.e.
8 (l, b_local) pairs per core. Each core returns a [4, 1088] block.
Measured: ~21.1-21.8us HW exec (baseline fp32 version: 26.9us); ~12.4us of
that is fixed harness overhead (entry consts + exit barrier/sem-clears).
"""

from contextlib import ExitStack

import numpy as np
import ml_dtypes

import concourse.bass as bass
from concourse import bacc, mybir
from concourse.bass_utils import run_bass_kernel_spmd

EPS = 1e-5
L, B, N, C = 4, 16, 1024, 64
P = 128            # SBUF partitions
T = N // P         # 8 agent sub-rows per partition
NCORES = 8
BPC = B // NCORES  # b per core
NPAIR = L * BPC    # (l, b_local) pairs per core

DMA_CHUNKS = [(0, 1), (1, 3), (3, 5), (5, 8)]  # chunk 0 on the scalar ring
GROUPS = [[0, 1], [2, 3], [4, 5], [6], [7]]    # pairs per recip/weights group
NG = len(GROUPS)
ACT_SQ = (0, 1, 3, 5)      # squares on ACT (pair 7: accum ops)
GP_SQ = (2, 4, 6)          # squares on GpSimd

F32 = mybir.dt.float32
BF16 = mybir.dt.bfloat16
OUT_W = NPAIR * P + NPAIR * 8  # 1024 + 64


def _chunk_of(j):
    for k, (a, b) in enumerate(DMA_CHUNKS):
        if a <= j < b:
            return k
    raise ValueError(j)


def _group_of(j):
    for g, pairs in enumerate(GROUPS):
        if j in pairs:
            return g, pairs.index(j)
    raise ValueError(j)


def build_nc() -> bass.Bass:
    nc = bacc.Bacc("TRN2", target_bir_lowering=False, debug=False, num_devices=NCORES)
    x = nc.declare_dram_parameter("x", [P, NPAIR, T, C], BF16, isOutput=False)
    out = nc.declare_dram_parameter("out", [4, OUT_W], F32, isOutput=True)

    one_f32 = nc.const_aps.aps[(F32, 1.0)]
    one_bf16 = nc.const_aps.aps[(BF16, 1.0)]

    ctx = ExitStack()
    with ctx:
        def sb(name, shape, dtype=F32):
            return ctx.enter_context(nc.sbuf_tensor(name, shape, dtype))

        xb = sb("xb", [P, NPAIR, T, C], BF16)
        xsq = sb("xsq", [P, NPAIR, T, C])
        msq = sb("msq", [P, NPAIR, T])
        rsq = sb("rsq", [P, NPAIR, T])
        W = sb("W", [P, NPAIR, 4, 4], BF16)   # (tt, [r,r,r2,r2])
        scr = sb("scr", [P, 1])
        stage = sb("stage", [4, OUT_W])
        psum_s = [
            ctx.enter_context(nc.psum_tensor(f"psum_s{h}", [4, 2 * P], F32))
            for h in range(3)
        ] + [
            ctx.enter_context(nc.psum_tensor(f"psum_t{h}", [4, P], F32))
            for h in range(2)
        ]
        psum_pq = ctx.enter_context(nc.psum_tensor("psum_pq", [1, NPAIR * 8], F32))

        s_dma = [nc.alloc_semaphore(f"s_dma{k}") for k in range(len(DMA_CHUNKS))]
        s_sqa = nc.alloc_semaphore("s_sqa")    # ACT squares done (ordered)
        s_sqg = nc.alloc_semaphore("s_sqg")    # GpSimd squares done (ordered)
        s_rsq = nc.alloc_semaphore("s_rsq")    # DVE reciprocal done (per group)
        s_w = nc.alloc_semaphore("s_w")        # r weights ready (per group)
        s_w2 = nc.alloc_semaphore("s_w2")      # r^2 weights ready (per group)
        s_pe = nc.alloc_semaphore("s_pe")      # matmul progress (1..5)
        s_st = nc.alloc_semaphore("s_st")      # DVE staging copies (1..3)
        s_sta = nc.alloc_semaphore("s_sta")    # ACT staging copies (1..2)
        s_dmo = nc.alloc_semaphore("s_dmo")    # out DMA receipts
        s_dve = nc.alloc_semaphore("s_dve")    # DVE same-engine RAW chain
        sems = s_dma + [s_sqa, s_sqg, s_rsq, s_w, s_w2, s_pe, s_st, s_sta,
                        s_dmo, s_dve]

        with nc.Block() as block:

            @block.sync
            def _(sync):
                for k, (a, b) in enumerate(DMA_CHUNKS):
                    sync.dma_start(
                        out=xb[:, a:b], in_=x[:, a:b]
                    ).then_inc(s_dma[k], 16)
                sync.wait_ge(s_sta, 2)
                sync.dma_start(out=out[:, 0:512], in_=stage[:, 0:512]).then_inc(
                    s_dmo, 16
                )

            @block.scalar
            def _(scalar):
                # dummy sqrt pulls the ACT table load off the critical path
                scalar.sqrt(scr[:], one_f32)

                def sq(j):
                    scalar.square(xsq[:, j], xb[:, j])._wait_ge(
                        s_dma[_chunk_of(j)], 16
                    ).then_inc(s_sqa)

                def weights(g):
                    pairs = GROUPS[g]
                    a, b = pairs[0], pairs[-1] + 1
                    scalar.activation(
                        W[:, a:b, :, 0:2],
                        rsq[:, a:b].rearrange("p j (tt u) -> p j tt u", u=2),
                        mybir.ActivationFunctionType.Sqrt,
                    )._wait_ge(s_rsq, g + 1).then_inc(s_w)

                sq(0)
                sq(1)
                sq(3)
                weights(0)
                sq(5)
                weights(1)
                # pair 7: per-sub-row square with fused accumulate writes its
                # per-agent msq without touching the saturated DVE queue
                for t in range(T):
                    acc = scalar.activation(
                        xsq[:, 7, t],
                        xb[:, 7, t],
                        mybir.ActivationFunctionType.Square,
                        accum_out=msq[:, 7, t : t + 1],
                    )
                    if t == 0:
                        acc._wait_ge(s_dma[3], 16)
                    acc.then_inc(s_sqa)
                weights(2)
                scalar.copy(
                    stage[:, 0:256], psum_s[0][:]
                )._wait_ge(s_pe, 1).then_inc(s_sta)
                weights(3)
                scalar.copy(
                    stage[:, 256:512], psum_s[1][:]
                )._wait_ge(s_pe, 2).then_inc(s_sta)
                weights(4)
                scalar.copy(
                    stage[:, 768:896], psum_s[3][:]
                )._wait_ge(s_pe, 4).then_inc(s_sta)
                scalar.copy(
                    stage[:, 896:1024], psum_s[4][:]
                )._wait_ge(s_pe, 5).then_inc(s_sta)
                # out-DMA B issues here, right behind its last staging copy
                # (HWDGE from this queue does not implicitly order after the
                # engine's own in-flight writes - wait on the copy sems)
                scalar.wait_ge(s_st, 2)
                scalar.wait_ge(s_sqg, 1)
                scalar.wait_ge(s_sta, 4)
                scalar.dma_start(
                    out=out[:, 512:OUT_W], in_=stage[:, 512:OUT_W]
                ).then_inc(s_dmo, 16)

            @block.gpsimd
            def _(gpsimd):
                # rows 1-3 of the pq slot are never written; zero them so the
                # out-DMA reads defined bytes
                gpsimd.memset(stage[:, NPAIR * P : OUT_W], 0.0).then_inc(s_sqg)

                def sq(j):
                    gpsimd.tensor_mul(xsq[:, j], xb[:, j], xb[:, j])._wait_ge(
                        s_dma[_chunk_of(j)], 16
                    ).then_inc(s_sqg)

                def w2(g):
                    pairs = GROUPS[g]
                    a, b = pairs[0], pairs[-1] + 1
                    gpsimd.tensor_copy(
                        W[:, a:b, :, 2:4],
                        rsq[:, a:b].rearrange("p j (tt u) -> p j tt u", u=2),
                    )._wait_ge(s_rsq, g + 1).then_inc(s_w2)

                sq(2)
                sq(4)
                sq(6)
                w2(0)
                w2(1)
                w2(2)
                w2(3)
                w2(4)

            @block.vector
            def _(vector):
                nred = [0]

                def red(j):
                    r = vector.tensor_reduce(
                        out=msq[:, j],
                        in_=xsq[:, j],
                        axis=mybir.AxisListType.X,
                        op=mybir.AluOpType.add,
                    )
                    if j in GP_SQ:
                        r._wait_ge(s_sqg, GP_SQ.index(j) + 2)
                    else:
                        r._wait_ge(s_sqa, ACT_SQ.index(j) + 1)
                    r.then_inc(s_dve)
                    nred[0] += 1

                def recip(g):
                    pairs = GROUPS[g]
                    a, b = pairs[0], pairs[-1] + 1
                    r = vector.reciprocal_approx_fast(
                        out=rsq[:, a:b], in_=msq[:, a:b]
                    )
                    if pairs == [7]:
                        # msq[:, 7] comes from ACT's 4 squares + 8 accums
                        r._wait_ge(s_sqa, 12)
                    else:
                        r._wait_ge(s_dve, nred[0])
                    r.then_inc(s_rsq)

                red(0)
                red(1)
                recip(0)
                red(2)
                red(3)
                recip(1)
                red(4)
                red(5)
                recip(2)
                red(6)
                recip(3)
                recip(4)
                # staging copies for pairs 0-1, 4-5 and the pq row
                vector.tensor_copy(
                    stage[:, 512:768], psum_s[2][:]
                )._wait_ge(s_pe, 3).then_inc(s_st)
                vector.wait_ge(s_sqg, 1)
                vector.tensor_copy(
                    stage[0:1, NPAIR * P : OUT_W], psum_pq[:]
                )._wait_ge(s_pe, 6).then_inc(s_st)

            @block.tensor
            def _(tensor):
                def smm(j, inc=False):
                    g, _slot = _group_of(j)
                    tensor.wait_ge(s_w, g + 1)
                    tensor.wait_ge(s_w2, g + 1)
                    tensor.wait_ge(s_dma[_chunk_of(j)], 16)
                    for tt in range(T // 2):
                        ps = (
                            psum_s[j // 2][:, P * (j % 2) : P * (j % 2) + P]
                            if j < 6
                            else psum_s[3 + (j - 6)][:]
                        )
                        mm = tensor.matmul(
                            ps,
                            W[:, j, tt],
                            xb[:, j, 2 * tt : 2 * tt + 2, :],
                            start=(tt == 0),
                            stop=(tt == T // 2 - 1),
                        )
                        if inc and tt == T // 2 - 1:
                            mm.then_inc(s_pe)

                def pq(j, inc=False):
                    # t2 partial sums: ones^T @ r^2 columns
                    mm = tensor.matmul(
                        psum_pq[:, 8 * j : 8 * j + 8],
                        one_bf16,
                        W[:, j, :, 2:4],
                        start=True,
                        stop=True,
                    )
                    if inc:
                        mm.then_inc(s_pe)

                for j in range(7):
                    smm(j, inc=(j in (1, 3, 5, 6)))
                for j in range(7):
                    pq(j)
                smm(7, inc=True)
                pq(7, inc=True)

        # No final receipt wait or sem clears: the walrus postamble clears
        # every semaphore ~6us after the out-DMA receipt lands, and the
        # stream-end barrier chain gives the write several microseconds of
        # margin before the host reads the buffer.
        del sems

    nc.compile()
    return nc


_NC_CACHE = None


def _get_nc():
    global _NC_CACHE
    if _NC_CACHE is None:
        _NC_CACHE = build_nc()
    return _NC_CACHE


def _shard_inputs(x_full: np.ndarray):
    """Full [L, B, N, C] fp32 -> per-core [P, NPAIR, T, C] bf16 blocks."""
    in_maps = []
    for k in range(NCORES):
        shard = x_full[:, BPC * k : BPC * (k + 1)].reshape(NPAIR, P, T, C)
        shard = np.ascontiguousarray(shard.transpose(1, 0, 2, 3)).astype(
            ml_dtypes.bfloat16
        )
        in_maps.append({"x": shard})
    return in_maps


def run_cores(x_full: np.ndarray, trace: bool = False):
    nc = _get_nc()
    in_maps = _shard_inputs(np.asarray(x_full))
    res = run_bass_kernel_spmd(nc, in_maps, list(range(NCORES)), trace=trace)
    outs = [res.results[k]["out"] for k in range(NCORES)]
    return outs, res


def reduce_host(outs) -> np.ndarray:
    total = 0.0
    for blk in outs:
        blk = blk.astype(np.float64)
        for j in range(NPAIR):
            s = blk[0, P * j : P * j + 64] + blk[1, P * j + 64 : P * j + 128]
            s2 = blk[2, P * j : P * j + 64] + blk[3, P * j + 64 : P * j + 128]
            t2 = blk[0, NPAIR * P + 8 * j : NPAIR * P + 8 * j + 8].sum()
            S0 = np.dot(s, s) - float(N)
            S1 = np.dot(s2, s2) - t2
            total += S0 - EPS * S1
    loss = total / (N * (N - 1)) / B
    return np.array(loss, dtype=np.float32)


def kernel(updated_agents: np.ndarray) -> np.ndarray:
    outs, _ = run_cores(np.asarray(updated_agents))
    return reduce_host(outs)


# revision 31
# speedup vs baseline: 1.0803x; 1.0803x over previous
"""Trainium2 Bass kernel for the AgentLoss problem (raw bacc, manual sems).

Math: for each (l, b) the reference computes the masked cosine-similarity sum
    S = sum_{i != j} <x_i, x_j> / (|x_i| |x_j| + EPS)
over n=1024 agents with c=64 channels, then loss = sum_l mean_b S / (n(n-1)).

Since EPS (1e-5) is tiny vs |x_i||x_j| ~ 64, expand
    1/(m_i m_j + EPS) = r_i r_j - EPS r_i^2 r_j^2 + O(EPS^2),  r_i = 1/m_i
which makes the double sum separable:
    S ~= (|sum_i x_i r_i|^2 - sum_i msq_i r_i^2)
         - EPS * (|sum_i x_i r_i^2|^2 - sum_i msq_i r_i^4)
with sum_i msq_i r_i^2 ~= n (fp32 recip, exact to ~1e-7) and
sum_i msq_i r_i^4 ~= sum_i r_i^2 (= t2, via a ones-stationary matmul).

The device side runs in bf16: the host pre-casts the input (cosine
similarity is scale-free and the loss averages ~16M sims, so the cast
costs ~3e-3 relative error - well under the 2e-2 gate), which halves HBM
traffic and lets the PE stream matmuls at full rate instead of fp32's
LOW/HIGH half-rate split.  Pipeline per (l, b) pair:

  in-DMA (4 chunks, sizes 1/2/2/3 pairs, one sem each - per-chunk sems are
  required because concurrent DMAs interleave their 16 per-engine sem incs)
  -> square: ACT pairs {0,1,3,5,7} / GpSimd {2,4,6} (xsq fp32)
  -> segmented reduce to per-agent msq: DVE tensor_reduce (the critical
     ~5.4us chain; nothing else on the chip can reduce a free axis)
  -> r^2 = 1/msq: DVE RECIPROCAL_APPROX_FAST custom op (~51 ULP, 5x faster
     than iterative divide), fp32 into rsq
  -> weights: ACT sqrt casts r to bf16, GpSimd copy casts r^2 to bf16,
     both into the [tt, (r,r,r2,r2)] stationary tile W
  -> thin bf16 matmuls contract the agent axis, 2 sub-rows x {r, r^2} per
     matmul (N=128 moving, half-garbage output the host discards); pairs
     6/7 write separate PSUM banks so the last staging copies wait only on
     their own pair (reading a bank that another accumulation group is
     mid-flight in is an NRT_EXEC_UNIT_UNRECOVERABLE on HW)
  -> staging copies split ACT/DVE, 2 out-DMAs.

Groups (2,2,2,1,1) drain the recip/sqrt ladder through single pairs at the
end.  A dummy sqrt up front pulls the ACT table load into the DMA phase.
No final receipt wait or semaphore clears: the framework postamble clears
all 253 sems (~7us) after the out-DMA receipt lands, giving the write a
multi-microsecond margin before stream end.  Host does the final ~2k-flop
combine in float64.

Sharding: data-parallel over batch b - core k takes b in {2k, 2k+1}, i.e.
8 (l, b_local) pairs per core. Each core returns a [4, 1088] block.
Measured: ~21.1-21.8us HW exec (baseline fp32 version: 26.9us); ~12.4us of
that is fixed harness overhead (entry consts + exit barrier/sem-clears).
"""

from contextlib import ExitStack

import numpy as np
import ml_dtypes

import concourse.bass as bass
from concourse import bacc, mybir
from concourse.bass_utils import run_bass_kernel_spmd

EPS = 1e-5
L, B, N, C = 4, 16, 1024, 64
P = 128            # SBUF partitions
T = N // P         # 8 agent sub-rows per partition
NCORES = 8
BPC = B // NCORES  # b per core
NPAIR = L * BPC    # (l, b_local) pairs per core

DMA_CHUNKS = [(0, 1), (1, 3), (3, 5), (5, 8)]  # chunk 0 on the scalar ring
GROUPS = [[0, 1], [2, 3], [4, 5], [6], [7]]    # pairs per recip/weights group
NG = len(GROUPS)
ACT_SQ = (0, 1, 3, 5, 7)   # squares on ACT
GP_SQ = (2, 4, 6)          # squares on GpSimd

F32 = mybir.dt.float32
BF16 = mybir.dt.bfloat16
OUT_W = NPAIR * P + NPAIR * 8  # 1024 + 64


def _chunk_of(j):
    for k, (a, b) in enumerate(DMA_CHUNKS):
        if a <= j < b:
            return k
    raise ValueError(j)


def _group_of(j):
    for g, pairs in enumerate(GROUPS):
        if j in pairs:
            return g, pairs.index(j)
    raise ValueError(j)


def build_nc() -> bass.Bass:
    nc = bacc.Bacc("TRN2", target_bir_lowering=False, debug=False, num_devices=NCORES)
    x = nc.declare_dram_parameter("x", [P, NPAIR, T, C], BF16, isOutput=False)
    out = nc.declare_dram_parameter("out", [4, OUT_W], F32, isOutput=True)

    one_f32 = nc.const_aps.aps[(F32, 1.0)]
    one_bf16 = nc.const_aps.aps[(BF16, 1.0)]

    ctx = ExitStack()
    with ctx:
        def sb(name, shape, dtype=F32):
            return ctx.enter_context(nc.sbuf_tensor(name, shape, dtype))

        xb = sb("xb", [P, NPAIR, T, C], BF16)
        xsq = sb("xsq", [P, NPAIR, T, C])
        msq = sb("msq", [P, NPAIR, T])
        rsq = sb("rsq", [P, NPAIR, T])
        W = sb("W", [P, NPAIR, 4, 4], BF16)   # (tt, [r,r,r2,r2])
        scr = sb("scr", [P, 1])
        stage = sb("stage", [4, OUT_W])
        psum_s = [
            ctx.enter_context(nc.psum_tensor(f"psum_s{h}", [4, 2 * P], F32))
            for h in range(3)
        ] + [
            ctx.enter_context(nc.psum_tensor(f"psum_t{h}", [4, P], F32))
            for h in range(2)
        ]
        psum_pq = ctx.enter_context(nc.psum_tensor("psum_pq", [1, NPAIR * 8], F32))

        s_dma = [nc.alloc_semaphore(f"s_dma{k}") for k in range(len(DMA_CHUNKS))]
        s_sqa = nc.alloc_semaphore("s_sqa")    # ACT squares done (ordered)
        s_sqg = nc.alloc_semaphore("s_sqg")    # GpSimd squares done (ordered)
        s_rsq = nc.alloc_semaphore("s_rsq")    # DVE reciprocal done (per group)
        s_w = nc.alloc_semaphore("s_w")        # r weights ready (per group)
        s_w2 = nc.alloc_semaphore("s_w2")      # r^2 weights ready (per group)
        s_pe = nc.alloc_semaphore("s_pe")      # matmul progress (1..5)
        s_st = nc.alloc_semaphore("s_st")      # DVE staging copies (1..3)
        s_sta = nc.alloc_semaphore("s_sta")    # ACT staging copies (1..2)
        s_dmo = nc.alloc_semaphore("s_dmo")    # out DMA receipts
        s_dve = nc.alloc_semaphore("s_dve")    # DVE same-engine RAW chain
        sems = s_dma + [s_sqa, s_sqg, s_rsq, s_w, s_w2, s_pe, s_st, s_sta,
                        s_dmo, s_dve]

        with nc.Block() as block:

            @block.sync
            def _(sync):
                for k, (a, b) in enumerate(DMA_CHUNKS):
                    sync.dma_start(
                        out=xb[:, a:b], in_=x[:, a:b]
                    ).then_inc(s_dma[k], 16)
                sync.wait_ge(s_sta, 2)
                sync.dma_start(out=out[:, 0:512], in_=stage[:, 0:512]).then_inc(
                    s_dmo, 16
                )
                sync.wait_ge(s_st, 2)
                sync.wait_ge(s_sta, 4)
                sync.wait_ge(s_sqg, 1)
                sync.dma_start(
                    out=out[:, 512:OUT_W], in_=stage[:, 512:OUT_W]
                ).then_inc(s_dmo, 16)

            @block.scalar
            def _(scalar):
                # dummy sqrt pulls the ACT table load off the critical path
                scalar.sqrt(scr[:], one_f32)

                def sq(j):
                    scalar.square(xsq[:, j], xb[:, j])._wait_ge(
                        s_dma[_chunk_of(j)], 16
                    ).then_inc(s_sqa)

                def weights(g):
                    pairs = GROUPS[g]
                    a, b = pairs[0], pairs[-1] + 1
                    scalar.activation(
                        W[:, a:b, :, 0:2],
                        rsq[:, a:b].rearrange("p j (tt u) -> p j tt u", u=2),
                        mybir.ActivationFunctionType.Sqrt,
                    )._wait_ge(s_rsq, g + 1).then_inc(s_w)

                sq(0)
                sq(1)
                sq(3)
                weights(0)
                sq(5)
                weights(1)
                sq(7)
                weights(2)
                scalar.copy(
                    stage[:, 0:256], psum_s[0][:]
                )._wait_ge(s_pe, 1).then_inc(s_sta)
                weights(3)
                scalar.copy(
                    stage[:, 256:512], psum_s[1][:]
                )._wait_ge(s_pe, 2).then_inc(s_sta)
                weights(4)
                scalar.copy(
                    stage[:, 768:896], psum_s[3][:]
                )._wait_ge(s_pe, 4).then_inc(s_sta)
                scalar.copy(
                    stage[:, 896:1024], psum_s[4][:]
                )._wait_ge(s_pe, 5).then_inc(s_sta)

            @block.gpsimd
            def _(gpsimd):
                # rows 1-3 of the pq slot are never written; zero them so the
                # out-DMA reads defined bytes
                gpsimd.memset(stage[:, NPAIR * P : OUT_W], 0.0).then_inc(s_sqg)

                def sq(j):
                    gpsimd.tensor_mul(xsq[:, j], xb[:, j], xb[:, j])._wait_ge(
                        s_dma[_chunk_of(j)], 16
                    ).then_inc(s_sqg)

                def w2(g):
                    pairs = GROUPS[g]
                    a, b = pairs[0], pairs[-1] + 1
                    gpsimd.tensor_copy(
                        W[:, a:b, :, 2:4],
                        rsq[:, a:b].rearrange("p j (tt u) -> p j tt u", u=2),
                    )._wait_ge(s_rsq, g + 1).then_inc(s_w2)

                sq(2)
                sq(4)
                sq(6)
                w2(0)
                w2(1)
                w2(2)
                w2(3)
                w2(4)

            @block.vector
            def _(vector):
                nred = [0]

                def red(j):
                    r = vector.tensor_reduce(
                        out=msq[:, j],
                        in_=xsq[:, j],
                        axis=mybir.AxisListType.X,
                        op=mybir.AluOpType.add,
                    )
                    if j in GP_SQ:
                        r._wait_ge(s_sqg, GP_SQ.index(j) + 2)
                    else:
                        r._wait_ge(s_sqa, ACT_SQ.index(j) + 1)
                    r.then_inc(s_dve)
                    nred[0] += 1

                def recip(g):
                    pairs = GROUPS[g]
                    a, b = pairs[0], pairs[-1] + 1
                    vector.reciprocal_approx_fast(
                        out=rsq[:, a:b], in_=msq[:, a:b]
                    )._wait_ge(s_dve, nred[0]).then_inc(s_rsq)

                red(0)
                red(1)
                recip(0)
                red(2)
                red(3)
                recip(1)
                red(4)
                red(5)
                recip(2)
                red(6)
                recip(3)
                red(7)
                recip(4)
                # staging copies for pairs 0-1, 4-5 and the pq row
                vector.tensor_copy(
                    stage[:, 512:768], psum_s[2][:]
                )._wait_ge(s_pe, 3).then_inc(s_st)
                vector.wait_ge(s_sqg, 1)
                vector.tensor_copy(
                    stage[0:1, NPAIR * P : OUT_W], psum_pq[:]
                )._wait_ge(s_pe, 6).then_inc(s_st)

            @block.tensor
            def _(tensor):
                def smm(j, inc=False):
                    g, _slot = _group_of(j)
                    tensor.wait_ge(s_w, g + 1)
                    tensor.wait_ge(s_w2, g + 1)
                    tensor.wait_ge(s_dma[_chunk_of(j)], 16)
                    for tt in range(T // 2):
                        ps = (
                            psum_s[j // 2][:, P * (j % 2) : P * (j % 2) + P]
                            if j < 6
                            else psum_s[3 + (j - 6)][:]
                        )
                        mm = tensor.matmul(
                            ps,
                            W[:, j, tt],
                            xb[:, j, 2 * tt : 2 * tt + 2, :],
                            start=(tt == 0),
                            stop=(tt == T // 2 - 1),
                        )
                        if inc and tt == T // 2 - 1:
                            mm.then_inc(s_pe)

                def pq(j, inc=False):
                    # t2 partial sums: ones^T @ r^2 columns
                    mm = tensor.matmul(
                        psum_pq[:, 8 * j : 8 * j + 8],
                        one_bf16,
                        W[:, j, :, 2:4],
                        start=True,
                        stop=True,
                    )
                    if inc:
                        mm.then_inc(s_pe)

                for j in range(7):
                    smm(j, inc=(j in (1, 3, 5, 6)))
                for j in range(7):
                    pq(j)
                smm(7, inc=True)
                pq(7, inc=True)

        # No final receipt wait or sem clears: the walrus postamble clears
        # every semaphore ~6us after the out-DMA receipt lands, and the
        # stream-end barrier chain gives the write several microseconds of
        # margin before the host reads the buffer.
        del sems

    nc.compile()
    return nc


_NC_CACHE = None


def _get_nc():
    global _NC_CACHE
    if _NC_CACHE is None:
        _NC_CACHE = build_nc()
    return _NC_CACHE


def _shard_inputs(x_full: np.ndarray):
    """Full [L, B, N, C] fp32 -> per-core [P, NPAIR, T, C] bf16 blocks."""
    in_maps = []
    for k in range(NCORES):
        shard = x_full[:, BPC * k : BPC * (k + 1)].reshape(NPAIR, P, T, C)
        shard = np.ascontiguousarray(shard.transpose(1, 0, 2, 3)).astype(
            ml_dtypes.bfloat16
        )
        in_maps.append({"x": shard})
    return in_maps


def run_cores(x_full: np.ndarray, trace: bool = False):
    nc = _get_nc()
    in_maps = _shard_inputs(np.asarray(x_full))
    res = run_bass_kernel_spmd(nc, in_maps, list(range(NCORES)), trace=trace)
    outs = [res.results[k]["out"] for k in range(NCORES)]
    return outs, res


def reduce_host(outs) -> np.ndarray:
    total = 0.0
    for blk in outs:
        blk = blk.astype(np.float64)
        for j in range(NPAIR):
            s = blk[0, P * j : P * j + 64] + blk[1, P * j + 64 : P * j + 128]
            s2 = blk[2, P * j : P * j + 64] + blk[3, P * j + 64 : P * j + 128]
            t2 = blk[0, NPAIR * P + 8 * j : NPAIR * P + 8 * j + 8].sum()
            S0 = np.dot(s, s) - float(N)
            S1 = np.dot(s2, s2) - t2
            total += S0 - EPS * S1
    loss = total / (N * (N - 1)) / B
    return np.array(loss, dtype=np.float32)


def kernel(updated_agents: np.ndarray) -> np.ndarray:
    outs, _ = run_cores(np.asarray(updated_agents))
    return reduce_host(outs)
